# revision 33
# baseline (speedup 1.0000x reference)
"""Trainium2 Bass kernel for an attention block (nn_AttentionBlock).

Reference computation (per batch element b of 16, c=512 channels, s=32*32=1024
tokens, 8 heads x 64 dim):
    xs   = x[b].reshape(c, s).T                  # [s, c]
    qkv  = xs @ W_proj + b_proj                  # [s, 1536]
    q,k,v per head; logits = q @ k.T * 1/8; p = softmax(logits, over keys)
    res  = p @ v (concat heads)                  # [s, 512]
    out  = res @ W_out + b_out + xs              # [s, c]
    y[b] = out.T.reshape(c, 32, 32)

Sharding: pure data-parallel over batch; each of the 8 cores processes 2
batch elements end-to-end (no collectives).

Per-core layout strategy (everything kept feature-major so no transposes are
ever needed):
  - x arrives as [c, s] which is exactly xs^T.
  - qT/kT are computed transposed ([feature, token]) with W as the stationary
    matmul operand and x streamed.
  - v is computed token-major ([token, feature]) with x as the stationary
    operand, augmented with a ones column per head so the p@v matmul also
    produces the softmax denominator row.
  - logits are computed transposed ([key, query]) so softmax needs no
    cross-partition max/sum: exp is elementwise (safe without max subtraction:
    |logits*scale| < ~6) and the denominator comes out of the PV matmul.
  - attention output lands as res^T ([feature, token]) which directly feeds
    the final projection with W_out stationary; the residual is added in
    [c, s] layout from a separate fp32 copy of x, and the output DMA'd back
    with no transpose.

Matmuls run in bf16 (1 cycle/row on the PE; fp32 is 4 cycles/row and fp32r
2 cycles/row measured). All accumulation is fp32 in PSUM and the residual
path is pure fp32, so the overall error stays in the few-1e-3 range.
"""

import sys

for _p in ("/opt/trn_rl_repo",):
    if _p not in sys.path:
        sys.path.insert(0, _p)

from contextlib import ExitStack

import ml_dtypes
import numpy as np

import concourse.bass as bass
import concourse.tile as tile
from concourse import bacc, mybir
from concourse.bass_utils import run_bass_kernel_spmd

dt = mybir.dt
AF = mybir.ActivationFunctionType
ALU = mybir.AluOpType

N_CORES = 8
B = 16
B_LOC = B // N_CORES  # images per core
C = 512  # channels
S = 1024  # tokens (32*32)
H = 8  # heads
D = 64  # dim per head
INNER = H * D  # 512
SCALE = D**-0.5

P = 128  # partitions
SH = S // 512  # 2 halves of the token dim (N=512 matmuls)
CT = C // P  # 4 channel tiles
TT = S // P  # 8 token tiles
FQK = 2 * INNER  # 1024 q+k features
QKT = FQK // P  # 8 qk feature tiles
DA = D + 1  # 65: v columns per head incl. ones column

MM = dt.bfloat16  # matmul operand dtype
NP_MM = ml_dtypes.bfloat16


def _build_nc():
    nc = bacc.Bacc("TRN2", target_bir_lowering=False, debug=False, num_devices=N_CORES)

    xin = nc.dram_tensor("xin", [B_LOC, C, S], MM, kind="ExternalInput").ap()
    xinf = nc.dram_tensor("xinf", [B_LOC, C, S], dt.float32, kind="ExternalInput").ap()
    wqk = nc.dram_tensor("wqk", [C, FQK], MM, kind="ExternalInput").ap()
    wv = nc.dram_tensor("wv", [C, INNER], MM, kind="ExternalInput").ap()
    wout = nc.dram_tensor("wout", [INNER, C], MM, kind="ExternalInput").ap()
    bqk = nc.dram_tensor("bqk", [FQK], dt.float32, kind="ExternalInput").ap()
    bv = nc.dram_tensor("bv", [1, INNER], dt.float32, kind="ExternalInput").ap()
    bout = nc.dram_tensor("bout", [C], dt.float32, kind="ExternalInput").ap()
    y = nc.dram_tensor("y", [B_LOC, C, S], dt.float32, kind="ExternalOutput").ap()
    warm = nc.dram_tensor("warm", [1, 512], dt.float32, kind="ExternalOutput").ap()

    with tile.TileContext(nc) as tc:
        with ExitStack() as ctx:
            _body(ctx, tc, nc, xin, xinf, wqk, wv, wout, bqk, bv, bout, y, warm)

    nc.compile()
    return nc


def _body(ctx, tc, nc, xin, xinf, wqk, wv, wout, bqk, bv, bout, y, warm):
    f32 = dt.float32

    consts = ctx.enter_context(tc.tile_pool(name="consts", bufs=1))
    xp = ctx.enter_context(tc.tile_pool(name="xp", bufs=2))
    xfp = ctx.enter_context(tc.tile_pool(name="xfp", bufs=3))
    qkp = ctx.enter_context(tc.tile_pool(name="qkp", bufs=2))
    vp = ctx.enter_context(tc.tile_pool(name="vp", bufs=2))
    resp = ctx.enter_context(tc.tile_pool(name="resp", bufs=2))
    expp = ctx.enter_context(tc.tile_pool(name="expp", bufs=2))
    pvsp = ctx.enter_context(tc.tile_pool(name="pvsp", bufs=1))
    recp = ctx.enter_context(tc.tile_pool(name="recp", bufs=2))
    outp = ctx.enter_context(tc.tile_pool(name="outp", bufs=3))

    # one shared 1-bank psum rotation for projection + PV matmuls (4 bufs),
    # plus a double-buffered 2-bank chunk for the QK logits feeding exp
    ps_sm = ctx.enter_context(tc.tile_pool(name="ps_sm", bufs=4, space="PSUM"))
    ps_lg = ctx.enter_context(tc.tile_pool(name="ps_lg", bufs=2, space="PSUM"))

    # --- constants / weights (loaded once) ---
    wqk_sb = [
        consts.tile([P, FQK], MM, tag=f"wqk{k}", name=f"wqk_sb{k}") for k in range(CT)
    ]
    wv_sb = [
        consts.tile([P, INNER], MM, tag=f"wv{k}", name=f"wv_sb{k}") for k in range(CT)
    ]
    wout_sb = [
        consts.tile([P, C], MM, tag=f"wout{k}", name=f"wout_sb{k}") for k in range(CT)
    ]
    x0_sb = [
        xp.tile([P, S], MM, tag=f"x{k}", name=f"x_sb_0_{k}") for k in range(CT)
    ]
    for k in range(CT):
        nc.sync.dma_start(x0_sb[k][:], xin[0, k * P : (k + 1) * P, :])
    for j in (0, QKT // 2):  # only the wqk blocks the first qkT units touch
        for k in range(CT):
            nc.sync.dma_start(
                wqk_sb[k][:, j * P : (j + 1) * P], wqk[k * P : (k + 1) * P, j * P : (j + 1) * P]
            )
    for k in range(CT):
        nc.sync.dma_start(wv_sb[k][:], wv[k * P : (k + 1) * P, :])
    for j in range(QKT):
        if j in (0, QKT // 2):
            continue
        for k in range(CT):
            nc.sync.dma_start(
                wqk_sb[k][:, j * P : (j + 1) * P], wqk[k * P : (k + 1) * P, j * P : (j + 1) * P]
            )
    bqk_sb = consts.tile([P, QKT], f32, tag="bqk")
    bqk_t = bqk.rearrange("(t p) -> t p", p=P)
    for j in range(QKT):
        nc.sync.dma_start(bqk_sb[:, j : j + 1], bqk_t[j, :])
    bout_sb = consts.tile([P, CT], f32, tag="bout")
    bout_t = bout.rearrange("(t p) -> t p", p=P)
    for j in range(CT):
        nc.sync.dma_start(bout_sb[:, j : j + 1], bout_t[j, :])
    bv_sb = consts.tile([1, INNER], f32, tag="bv")
    nc.sync.dma_start(bv_sb[:], bv[:])
    bv_bc = consts.tile([P, INNER], f32, tag="bvbc")
    nc.gpsimd.partition_broadcast(bv_bc[:], bv_sb[:])
    ones_col = consts.tile([1, P], MM, tag="ones")
    nc.vector.memset(ones_col[:], 1.0)
    ones8 = consts.tile([P, H], MM, tag="ones8")
    nc.vector.memset(ones8[:], 1.0)

    # PE warmup: ~10us of dummy matmuls while the first DMAs land, so the
    # HAM clock-gate reaches 2.4 GHz before the real work starts. The dummy
    # DRAM output keeps it from being dead-code eliminated.
    wscr = consts.tile([1, 512], MM, tag="wscr")
    nc.vector.memset(wscr[:], 1.0)
    wps = ps_sm.tile([P, 512], f32, tag="sm", name="warm_psum")
    NWARM = 16
    for i in range(NWARM):
        nc.tensor.matmul(
            wps[:], ones_col[:], wscr[:], start=(i == 0), stop=(i == NWARM - 1)
        )
    wsb = consts.tile([1, 512], f32, tag="wsb")
    nc.vector.tensor_copy(wsb[:], wps[0:1, :])
    nc.sync.dma_start(warm[:], wsb[:])

    for k in range(CT):
        nc.sync.dma_start(wout_sb[k][:], wout[k * P : (k + 1) * P, :])

    st = {}  # per-image tiles

    def emit_x(im):
        if im == 0:
            st[0] = {"x": x0_sb}
            return
        x_sb = [
            xp.tile([P, S], MM, tag=f"x{k}", name=f"x_sb_{im}_{k}") for k in range(CT)
        ]
        for k in range(CT):
            nc.sync.dma_start(x_sb[k][:], xin[im, k * P : (k + 1) * P, :])
        st[im] = {"x": x_sb}

    def alloc_qk(im):
        st[im]["qk"] = [
            qkp.tile([P, S], MM, tag=f"qk{j}", name=f"qk_sb_{im}_{j}")
            for j in range(QKT)
        ]

    def alloc_v(im):
        st[im]["v"] = [
            vp.tile([P, H * DA], MM, tag=f"v{j}", name=f"v_sb_{im}_{j}")
            for j in range(TT)
        ]

    def qkT_unit(im, j, sh):
        def _e():
            x_sb = st[im]["x"]
            psum = ps_sm.tile([P, 512], f32, tag="sm")
            for k in range(CT):
                nc.tensor.matmul(
                    psum[:],
                    wqk_sb[k][:, j * P : (j + 1) * P],
                    x_sb[k][:, sh * 512 : (sh + 1) * 512],
                    start=(k == 0),
                    stop=(k == CT - 1),
                )
            nc.vector.tensor_scalar_add(
                st[im]["qk"][j][:, sh * 512 : (sh + 1) * 512],
                psum[:],
                bqk_sb[:, j : j + 1],
            )

        return _e

    def v_unit(im, j):
        def _e():
            x_sb = st[im]["x"]
            psum = ps_sm.tile([P, INNER], f32, tag="sm")
            for k in range(CT):
                nc.tensor.matmul(
                    psum[:],
                    x_sb[k][:, j * P : (j + 1) * P],
                    wv_sb[k][:],
                    start=(k == 0),
                    stop=(k == CT - 1),
                )
            v3 = st[im]["v"][j][:].rearrange("p (h d) -> p h d", h=H)
            nc.vector.tensor_tensor(
                v3[:, :, 0:D],
                psum[:].rearrange("p (h d) -> p h d", h=H),
                bv_bc[:].rearrange("p (h d) -> p h d", h=H),
                op=ALU.add,
            )
            nc.vector.tensor_copy(
                v3[:, :, D : D + 1], ones8[:].rearrange("p (h o) -> p h o", o=1)
            )

        return _e

    def fin_unit(im, j, sh):
        def _e():
            psum = ps_sm.tile([P, 512], f32, tag="sm")
            for k in range(CT):
                nc.tensor.matmul(
                    psum[:],
                    wout_sb[k][:, j * P : (j + 1) * P],
                    st[im]["res"][k][:, sh * 512 : (sh + 1) * 512],
                    start=(k == 0),
                    stop=(k == CT - 1),
                )
            xf = xfp.tile([P, 512], f32, tag="xf")
            nc.sync.dma_start(
                xf[:], xinf[im, j * P : (j + 1) * P, sh * 512 : (sh + 1) * 512]
            )
            o = outp.tile([P, 512], f32, tag="out")
            nc.vector.scalar_tensor_tensor(
                o[:], psum[:], bout_sb[:, j : j + 1], xf[:], op0=ALU.add, op1=ALU.add
            )
            nc.sync.dma_start(
                y[im, j * P : (j + 1) * P, sh * 512 : (sh + 1) * 512], o[:]
            )

        return _e

    def pv_subunits(im, h, sh, ex5, hs):
        # 4 sub-units of 2 matmuls each; staging copy + denominator gather
        # attached to the last one
        units = []
        pv = [None]

        def _mk(j0):
            def _e():
                if j0 == 0:
                    pv[0] = ps_sm.tile([DA, 512], f32, tag="sm", name=f"pv_{im}_{h}_{sh}")
                for j in (j0, j0 + 1):
                    nc.tensor.matmul(
                        pv[0][:],
                        st[im]["v"][j][:, h * DA : (h + 1) * DA],
                        ex5[:, j, hs, :],
                        start=(j == 0),
                        stop=(j == TT - 1),
                    )
                if j0 == TT - 2:
                    nc.vector.tensor_copy(
                        st[im]["pvs"][h][:, sh * 512 : (sh + 1) * 512], pv[0][0:D, :]
                    )
                    dstage = recp.tile([1, 512], f32, tag="dstage")
                    nc.vector.tensor_copy(dstage[:], pv[0][D : D + 1, :])
                    i = (h % 2) * SH + sh
                    nc.sync.dma_start(st[im]["den"][h // 2][i : i + 1, :], dstage[:])
                    if h % 2 == 1 and sh == SH - 1:
                        normalize_pair(im, h // 2)

            return _e

        for j0 in range(0, TT, 2):
            units.append(_mk(j0))
        return units

    def normalize_pair(im, pi):
        rec = recp.tile([2 * SH, 512], f32, tag="rec", name=f"rec_{im}_{pi}")
        nc.vector.reciprocal_approx_fast(rec[:], st[im]["den"][pi][:])
        for hs in range(2):
            h = 2 * pi + hs
            po = hs * D
            r0 = recp.tile([1, S], f32, tag="r0")
            nc.sync.dma_start(r0[:], rec[hs * SH : hs * SH + SH, :])
            rec_b = recp.tile([D, S], f32, tag="recb")
            nc.gpsimd.partition_broadcast(rec_b[:], r0[:])
            nc.vector.tensor_tensor(
                st[im]["res"][pi][po : po + D, :],
                st[im]["pvs"][h][:],
                rec_b[:],
                op=ALU.mult,
            )

    xq = []  # cross-image fill units (fin of prev image, qkT/v of next)
    pvq = []  # PV fill units; carries across images so the last slot's PV
    # matmuls interleave with the next image's first QK chunks

    def attention(im):
        nonlocal pvq
        st[im]["pvs"] = [
            pvsp.tile([D, S], MM, tag=f"pvs{h}", name=f"pvs_sb_{im}_{h}")
            for h in range(H)
        ]
        st[im]["den"] = {}
        st[im]["res"] = [
            resp.tile([P, S], MM, tag=f"res{k}", name=f"res_sb_{im}_{k}")
            for k in range(CT)
        ]
        slot_idx = 0
        for pi in range(H // 2):
            q_tile = st[im]["qk"][pi]
            k_tile = st[im]["qk"][QKT // 2 + pi]
            if pi not in st[im]["den"]:
                st[im]["den"][pi] = recp.tile(
                    [2 * SH, 512], f32, tag="den", name=f"den_{im}_{pi}"
                )
            for sh in range(SH):
                ex = expp.tile([P, TT * 2 * 512], MM, tag="exp")
                ex5 = ex[:].rearrange("p (t g s) -> p t g s", t=TT, g=2)
                for j in range(TT):
                    pl = ps_lg.tile([P, 1024], f32, tag="lg")
                    for hs in range(2):  # head A (rows 0:64), head B (64:128)
                        po = hs * D
                        nc.tensor.matmul(
                            pl[:, hs * 512 : (hs + 1) * 512],
                            k_tile[po : po + D, j * P : (j + 1) * P],
                            q_tile[po : po + D, sh * 512 : (sh + 1) * 512],
                            start=True,
                            stop=True,
                        )
                    nc.scalar.activation(
                        ex5[:, j, :, :],
                        pl[:].rearrange("p (g s) -> p g s", g=2),
                        AF.Exp,
                        scale=SCALE,
                    )
                    if pvq:
                        pvq.pop(0)()
                    elif xq and (slot_idx > 0 or j % 2 == 1):
                        # slot 0 of an image has no PV fills, so every chunk
                        # pop would hit xq and pace the very first exps at PE
                        # speed; fill only alternate chunks there
                        xq.pop(0)()
                    # one extra fill per chosen chunk, never two adjacent:
                    # ACT's lookahead backlog (~2 chunks) absorbs a single
                    # ~1.6us insertion but starves on anything larger
                    if xq and (
                        j in (2, 4, 6)
                        if slot_idx == 0
                        else (j in (1, 4, 6) if slot_idx < 5 else j in (1, 3, 5, 7))
                    ):
                        xq.pop(0)()
                if slot_idx == 0:
                    # finish the fills slot 0 skipped (v units must all be
                    # emitted before the PV units queued below consume them)
                    for _ in range(min(2, len(xq))):
                        xq.pop(0)()
                slot_idx += 1
                for hs in range(2):
                    pvq += pv_subunits(im, 2 * pi + hs, sh, ex5, hs)
        while pvq:
            pvq.pop(0)()

    # --- prologue: image 0 projections (qk tiles 0 and 4 first so attention
    # can start early; the rest plus v are needed before the first PV) ---
    emit_x(0)
    alloc_qk(0)
    alloc_v(0)
    for j in (0, QKT // 2):
        for sh in range(SH):
            qkT_unit(0, j, sh)()
    for j in range(TT):
        xq.append(v_unit(0, j))
    for j in (1, 5, 2, 6, 3, 7):
        for sh in range(SH):
            xq.append(qkT_unit(0, j, sh))

    for im in range(B_LOC):
        if im + 1 < B_LOC:
            emit_x(im + 1)
            alloc_qk(im + 1)
            alloc_v(im + 1)
            for j in (0, 4):
                for sh in range(SH):
                    xq.append(qkT_unit(im + 1, j, sh))
            for j in range(TT):
                xq.append(v_unit(im + 1, j))
            for j in (1, 5):
                for sh in range(SH):
                    xq.append(qkT_unit(im + 1, j, sh))
        attention(im)
        if im + 1 == B_LOC:
            while pvq:
                pvq.pop(0)()
            while xq:
                xq.pop(0)()
        if im + 1 < B_LOC:
            # pair-2/3 qkT of the next image drains inside its own early
            # slots (2 guaranteed pops per slot reach each pair in time)
            for j in (2, 6, 3, 7):
                for sh in range(SH):
                    xq.append(qkT_unit(im + 1, j, sh))
        if im + 1 < B_LOC:
            for j in range(CT):
                for sh in range(SH):
                    xq.append(fin_unit(im, j, sh))
        else:
            for j in range(CT):
                for sh in range(SH):
                    fin_unit(im, j, sh)()


_NC_CACHE = None
TRACE = False  # set True before calling kernel() to capture an NTFF profile
LAST_RESULT = None


def _get_nc():
    global _NC_CACHE
    if _NC_CACHE is None:
        _NC_CACHE = _build_nc()
    return _NC_CACHE


def kernel(x, W_proj, b_proj, W_out, b_out):
    x = np.ascontiguousarray(np.asarray(x, dtype=np.float32))
    W_proj = np.asarray(W_proj, dtype=np.float32)
    b_proj = np.asarray(b_proj, dtype=np.float32)
    W_out = np.ascontiguousarray(np.asarray(W_out, dtype=np.float32))
    b_out = np.ascontiguousarray(np.asarray(b_out, dtype=np.float32))

    # host-side weight rearrangement: [c, h*(q64|k64|v64)] -> q-block | k-block, v-block
    w3 = W_proj.reshape(C, H, 3 * D)
    w_q = w3[:, :, 0:D].reshape(C, INNER)
    w_k = w3[:, :, D : 2 * D].reshape(C, INNER)
    w_v = np.ascontiguousarray(w3[:, :, 2 * D : 3 * D].reshape(C, INNER))
    w_qk = np.ascontiguousarray(np.concatenate([w_q, w_k], axis=1))
    b3 = b_proj.reshape(H, 3 * D)
    b_qk = np.ascontiguousarray(
        np.concatenate([b3[:, 0:D].reshape(-1), b3[:, D : 2 * D].reshape(-1)])
    )
    b_v = np.ascontiguousarray(b3[:, 2 * D : 3 * D].reshape(1, INNER))

    xs = x.reshape(B, C, S)
    xs_mm = xs.astype(NP_MM)
    w_qk = w_qk.astype(NP_MM)
    w_v = w_v.astype(NP_MM)
    w_out_mm = W_out.astype(NP_MM)

    nc = _get_nc()
    in_maps = []
    for i in range(N_CORES):
        in_maps.append(
            {
                "xin": np.ascontiguousarray(xs_mm[i * B_LOC : (i + 1) * B_LOC]),
                "xinf": np.ascontiguousarray(xs[i * B_LOC : (i + 1) * B_LOC]),
                "wqk": w_qk,
                "wv": w_v,
                "wout": w_out_mm,
                "bqk": b_qk,
                "bv": b_v,
                "bout": b_out,
            }
        )
    res = run_bass_kernel_spmd(nc, in_maps, list(range(N_CORES)), trace=TRACE)
    global LAST_RESULT
    LAST_RESULT = res
    out = np.concatenate([res.results[i]["y"] for i in range(N_CORES)], axis=0)
    return out.reshape(B, C, 32, 32)


if __name__ == "__main__":
    rng = np.random.default_rng(0)
    ins = {
        "x": rng.standard_normal((B, C, 32, 32), dtype=np.float32),
        "W_proj": rng.standard_normal((C, 3 * INNER), dtype=np.float32) * C**-0.5,
        "b_proj": rng.standard_normal((3 * INNER,), dtype=np.float32) * 0.01,
        "W_out": rng.standard_normal((INNER, C), dtype=np.float32) * INNER**-0.5,
        "b_out": rng.standard_normal((C,), dtype=np.float32) * 0.01,
    }
    out = kernel(**ins)
    print(out.shape, out.dtype)


# revision 35
# speedup vs baseline: 1.2616x; 1.2616x over previous
"""Trainium2 Bass kernel for an attention block (nn_AttentionBlock).

Reference computation (per batch element b of 16, c=512 channels, s=32*32=1024
tokens, 8 heads x 64 dim):
    xs   = x[b].reshape(c, s).T                  # [s, c]
    qkv  = xs @ W_proj + b_proj                  # [s, 1536]
    q,k,v per head; logits = q @ k.T * 1/8; p = softmax(logits, over keys)
    res  = p @ v (concat heads)                  # [s, 512]
    out  = res @ W_out + b_out + xs              # [s, c]
    y[b] = out.T.reshape(c, 32, 32)

Sharding: pure data-parallel over batch; each of the 8 cores processes 2
batch elements end-to-end (no collectives).

Per-core layout strategy (everything kept feature-major so no transposes are
ever needed):
  - x arrives as [c, s] which is exactly xs^T.
  - qT/kT are computed transposed ([feature, token]) with W as the stationary
    matmul operand and x streamed.
  - v is computed token-major ([token, feature]) with x as the stationary
    operand, augmented with a ones column per head so the p@v matmul also
    produces the softmax denominator row.
  - logits are computed transposed ([key, query]) so softmax needs no
    cross-partition max/sum: exp is elementwise (safe without max subtraction:
    |logits*scale| < ~6) and the denominator comes out of the PV matmul.
  - attention output lands as res^T ([feature, token]) which directly feeds
    the final projection with W_out stationary; the residual is added in
    [c, s] layout from a separate fp32 copy of x, and the output DMA'd back
    with no transpose.

Matmuls run in bf16 (1 cycle/row on the PE; fp32 is 4 cycles/row and fp32r
2 cycles/row measured). All accumulation is fp32 in PSUM and the residual
path is pure fp32, so the overall error stays in the few-1e-3 range.
"""

import sys

for _p in ("/opt/trn_rl_repo",):
    if _p not in sys.path:
        sys.path.insert(0, _p)

from contextlib import ExitStack

import ml_dtypes
import numpy as np

import concourse.bass as bass
import concourse.tile as tile
from concourse import bacc, mybir
from concourse.bass_utils import run_bass_kernel_spmd

dt = mybir.dt
AF = mybir.ActivationFunctionType
ALU = mybir.AluOpType

N_CORES = 8
B = 16
B_LOC = B // N_CORES  # images per core
C = 512  # channels
S = 1024  # tokens (32*32)
H = 8  # heads
D = 64  # dim per head
INNER = H * D  # 512
SCALE = D**-0.5

P = 128  # partitions
SH = S // 512  # 2 halves of the token dim (N=512 matmuls)
CT = C // P  # 4 channel tiles
TT = S // P  # 8 token tiles
FQK = 2 * INNER  # 1024 q+k features
QKT = FQK // P  # 8 qk feature tiles
DA = D + 1  # 65: v columns per head incl. ones column

MM = dt.bfloat16  # matmul operand dtype
NP_MM = ml_dtypes.bfloat16


def _build_nc():
    nc = bacc.Bacc("TRN2", target_bir_lowering=False, debug=False, num_devices=N_CORES)

    xin = nc.dram_tensor("xin", [B_LOC, C, S], MM, kind="ExternalInput").ap()
    xinf = nc.dram_tensor("xinf", [B_LOC, C, S], dt.float32, kind="ExternalInput").ap()
    wqk = nc.dram_tensor("wqk", [C, FQK], MM, kind="ExternalInput").ap()
    wv = nc.dram_tensor("wv", [C, INNER], MM, kind="ExternalInput").ap()
    wout = nc.dram_tensor("wout", [INNER, C], MM, kind="ExternalInput").ap()
    bqk = nc.dram_tensor("bqk", [FQK], dt.float32, kind="ExternalInput").ap()
    bv = nc.dram_tensor("bv", [1, INNER], dt.float32, kind="ExternalInput").ap()
    bout = nc.dram_tensor("bout", [C], dt.float32, kind="ExternalInput").ap()
    y = nc.dram_tensor("y", [B_LOC, C, S], dt.float32, kind="ExternalOutput").ap()
    warm = nc.dram_tensor("warm", [1, 512], dt.float32, kind="ExternalOutput").ap()

    with tile.TileContext(nc) as tc:
        with ExitStack() as ctx:
            _body(ctx, tc, nc, xin, xinf, wqk, wv, wout, bqk, bv, bout, y, warm)

    nc.compile()
    return nc


POS = [0, 2, 4, 6, 1, 3, 5, 7]  # column position of logical wqk block j


def prep_weights(W_proj, b_proj):
    """Host-side rearrangement: per-head q/k columns into a q-block|k-block
    matrix with its 128-column blocks permuted [0,4,1,5,2,6,3,7] (so the
    blocks the first qkT units need sit in the DMA'd-first columns), plus
    the v weight/bias blocks."""
    w3 = W_proj.reshape(C, H, 3 * D)
    w_q = w3[:, :, 0:D].reshape(C, INNER)
    w_k = w3[:, :, D : 2 * D].reshape(C, INNER)
    w_v = np.ascontiguousarray(w3[:, :, 2 * D : 3 * D].reshape(C, INNER))
    perm = [0, 4, 1, 5, 2, 6, 3, 7]
    w_qk = np.concatenate([w_q, w_k], axis=1)
    w_qk = np.ascontiguousarray(w_qk.reshape(C, 8, P)[:, perm, :].reshape(C, FQK))
    b3 = b_proj.reshape(H, 3 * D)
    b_qk = np.concatenate([b3[:, 0:D].reshape(-1), b3[:, D : 2 * D].reshape(-1)])
    b_qk = np.ascontiguousarray(b_qk.reshape(8, P)[perm, :].reshape(-1))
    b_v = np.ascontiguousarray(b3[:, 2 * D : 3 * D].reshape(1, INNER))
    return w_qk, w_v, b_qk, b_v


def _body(ctx, tc, nc, xin, xinf, wqk, wv, wout, bqk, bv, bout, y, warm):
    f32 = dt.float32

    consts = ctx.enter_context(tc.tile_pool(name="consts", bufs=1))
    xp = ctx.enter_context(tc.tile_pool(name="xp", bufs=2))
    xfp = ctx.enter_context(tc.tile_pool(name="xfp", bufs=3))
    qkp = ctx.enter_context(tc.tile_pool(name="qkp", bufs=2))
    vp = ctx.enter_context(tc.tile_pool(name="vp", bufs=2))
    resp = ctx.enter_context(tc.tile_pool(name="resp", bufs=2))
    expp = ctx.enter_context(tc.tile_pool(name="expp", bufs=2))
    pvsp = ctx.enter_context(tc.tile_pool(name="pvsp", bufs=1))
    recp = ctx.enter_context(tc.tile_pool(name="recp", bufs=2))
    outp = ctx.enter_context(tc.tile_pool(name="outp", bufs=3))

    # one shared 1-bank psum rotation for projection + PV matmuls (4 bufs),
    # plus a double-buffered 2-bank chunk for the QK logits feeding exp
    ps_sm = ctx.enter_context(tc.tile_pool(name="ps_sm", bufs=4, space="PSUM"))
    ps_lg = ctx.enter_context(tc.tile_pool(name="ps_lg", bufs=2, space="PSUM"))

    # --- constants / weights (loaded once) ---
    wqk_sb = [
        consts.tile([P, FQK], MM, tag=f"wqk{k}", name=f"wqk_sb{k}") for k in range(CT)
    ]
    wv_sb = [
        consts.tile([P, INNER], MM, tag=f"wv{k}", name=f"wv_sb{k}") for k in range(CT)
    ]
    wout_sb = [
        consts.tile([P, C], MM, tag=f"wout{k}", name=f"wout_sb{k}") for k in range(CT)
    ]
    x0_sb = [
        xp.tile([P, S], MM, tag=f"x{k}", name=f"x_sb_0_{k}") for k in range(CT)
    ]
    for k in range(CT):
        nc.sync.dma_start(x0_sb[k][:], xin[0, k * P : (k + 1) * P, :])
    for k in range(CT):  # leading columns hold the blocks qkT{0,4} need
        nc.sync.dma_start(
            wqk_sb[k][:, 0 : 2 * P], wqk[k * P : (k + 1) * P, 0 : 2 * P]
        )
    for k in range(CT):
        nc.sync.dma_start(wv_sb[k][:], wv[k * P : (k + 1) * P, :])
    for k in range(CT):
        nc.sync.dma_start(
            wqk_sb[k][:, 2 * P :], wqk[k * P : (k + 1) * P, 2 * P :]
        )
    bqk_sb = consts.tile([P, QKT], f32, tag="bqk")
    bqk_t = bqk.rearrange("(t p) -> t p", p=P)
    for j in range(QKT):
        nc.sync.dma_start(bqk_sb[:, j : j + 1], bqk_t[j, :])
    bout_sb = consts.tile([P, CT], f32, tag="bout")
    bout_t = bout.rearrange("(t p) -> t p", p=P)
    for j in range(CT):
        nc.sync.dma_start(bout_sb[:, j : j + 1], bout_t[j, :])
    bv_sb = consts.tile([1, INNER], f32, tag="bv")
    nc.sync.dma_start(bv_sb[:], bv[:])
    bv_bc = consts.tile([P, INNER], f32, tag="bvbc")
    nc.gpsimd.partition_broadcast(bv_bc[:], bv_sb[:])
    ones_col = consts.tile([1, P], MM, tag="ones")
    nc.vector.memset(ones_col[:], 1.0)
    ones8 = consts.tile([P, H], MM, tag="ones8")
    nc.vector.memset(ones8[:], 1.0)

    # PE warmup: ~10us of dummy matmuls while the first DMAs land, so the
    # HAM clock-gate reaches 2.4 GHz before the real work starts. The dummy
    # DRAM output keeps it from being dead-code eliminated.
    wscr = consts.tile([1, 512], MM, tag="wscr")
    nc.vector.memset(wscr[:], 1.0)
    wps = ps_sm.tile([P, 512], f32, tag="sm", name="warm_psum")
    NWARM = 16
    for i in range(NWARM):
        nc.tensor.matmul(
            wps[:], ones_col[:], wscr[:], start=(i == 0), stop=(i == NWARM - 1)
        )
    wsb = consts.tile([1, 512], f32, tag="wsb")
    nc.vector.tensor_copy(wsb[:], wps[0:1, :])
    nc.sync.dma_start(warm[:], wsb[:])

    for k in range(CT):
        nc.sync.dma_start(wout_sb[k][:], wout[k * P : (k + 1) * P, :])

    st = {}  # per-image tiles

    def emit_x(im):
        if im == 0:
            st[0] = {"x": x0_sb}
            return
        x_sb = [
            xp.tile([P, S], MM, tag=f"x{k}", name=f"x_sb_{im}_{k}") for k in range(CT)
        ]
        for k in range(CT):
            nc.sync.dma_start(x_sb[k][:], xin[im, k * P : (k + 1) * P, :])
        st[im] = {"x": x_sb}

    def alloc_qk(im):
        st[im]["qk"] = [
            qkp.tile([P, S], MM, tag=f"qk{j}", name=f"qk_sb_{im}_{j}")
            for j in range(QKT)
        ]

    def alloc_v(im):
        st[im]["v"] = [
            vp.tile([P, H * DA], MM, tag=f"v{j}", name=f"v_sb_{im}_{j}")
            for j in range(TT)
        ]

    def qkT_unit(im, j, sh):
        def _e():
            x_sb = st[im]["x"]
            psum = ps_sm.tile([P, 512], f32, tag="sm")
            p = POS[j]
            for k in range(CT):
                nc.tensor.matmul(
                    psum[:],
                    wqk_sb[k][:, p * P : (p + 1) * P],
                    x_sb[k][:, sh * 512 : (sh + 1) * 512],
                    start=(k == 0),
                    stop=(k == CT - 1),
                )
            nc.vector.tensor_scalar_add(
                st[im]["qk"][j][:, sh * 512 : (sh + 1) * 512],
                psum[:],
                bqk_sb[:, p : p + 1],
            )

        return _e

    def v_unit(im, j):
        def _e():
            x_sb = st[im]["x"]
            psum = ps_sm.tile([P, INNER], f32, tag="sm")
            for k in range(CT):
                nc.tensor.matmul(
                    psum[:],
                    x_sb[k][:, j * P : (j + 1) * P],
                    wv_sb[k][:],
                    start=(k == 0),
                    stop=(k == CT - 1),
                )
            v3 = st[im]["v"][j][:].rearrange("p (h d) -> p h d", h=H)
            nc.vector.tensor_tensor(
                v3[:, :, 0:D],
                psum[:].rearrange("p (h d) -> p h d", h=H),
                bv_bc[:].rearrange("p (h d) -> p h d", h=H),
                op=ALU.add,
            )
            nc.vector.tensor_copy(
                v3[:, :, D : D + 1], ones8[:].rearrange("p (h o) -> p h o", o=1)
            )

        return _e

    def fin_unit(im, j, sh):
        def _e():
            psum = ps_sm.tile([P, 512], f32, tag="sm")
            for k in range(CT):
                nc.tensor.matmul(
                    psum[:],
                    wout_sb[k][:, j * P : (j + 1) * P],
                    st[im]["res"][k][:, sh * 512 : (sh + 1) * 512],
                    start=(k == 0),
                    stop=(k == CT - 1),
                )
            xf = xfp.tile([P, 512], f32, tag="xf")
            nc.sync.dma_start(
                xf[:], xinf[im, j * P : (j + 1) * P, sh * 512 : (sh + 1) * 512]
            )
            o = outp.tile([P, 512], f32, tag="out")
            nc.vector.scalar_tensor_tensor(
                o[:], psum[:], bout_sb[:, j : j + 1], xf[:], op0=ALU.add, op1=ALU.add
            )
            nc.sync.dma_start(
                y[im, j * P : (j + 1) * P, sh * 512 : (sh + 1) * 512], o[:]
            )

        return _e

    def pv_subunits(im, h, sh, ex5, hs):
        # 4 sub-units of 2 matmuls each; staging copy + denominator gather
        # attached to the last one
        units = []
        pv = [None]

        def _mk(j0):
            def _e():
                if j0 == 0:
                    pv[0] = ps_sm.tile([DA, 512], f32, tag="sm", name=f"pv_{im}_{h}_{sh}")
                for j in (j0, j0 + 1):
                    nc.tensor.matmul(
                        pv[0][:],
                        st[im]["v"][j][:, h * DA : (h + 1) * DA],
                        ex5[:, j, hs, :],
                        start=(j == 0),
                        stop=(j == TT - 1),
                    )
                if j0 == TT - 2:
                    nc.vector.tensor_copy(
                        st[im]["pvs"][h][:, sh * 512 : (sh + 1) * 512], pv[0][0:D, :]
                    )
                    dstage = recp.tile([1, 512], f32, tag="dstage")
                    nc.vector.tensor_copy(dstage[:], pv[0][D : D + 1, :])
                    i = (h % 2) * SH + sh
                    nc.sync.dma_start(st[im]["den"][h // 2][i : i + 1, :], dstage[:])
                    if h % 2 == 1 and sh == SH - 1:
                        normalize_pair(im, h // 2)

            return _e

        for j0 in range(0, TT, 2):
            units.append(_mk(j0))
        return units

    def normalize_pair(im, pi):
        rec = recp.tile([2 * SH, 512], f32, tag="rec", name=f"rec_{im}_{pi}")
        nc.vector.reciprocal_approx_fast(rec[:], st[im]["den"][pi][:])
        for hs in range(2):
            h = 2 * pi + hs
            po = hs * D
            r0 = recp.tile([1, S], f32, tag="r0")
            nc.sync.dma_start(r0[:], rec[hs * SH : hs * SH + SH, :])
            rec_b = recp.tile([D, S], f32, tag="recb")
            nc.gpsimd.partition_broadcast(rec_b[:], r0[:])
            nc.vector.tensor_tensor(
                st[im]["res"][pi][po : po + D, :],
                st[im]["pvs"][h][:],
                rec_b[:],
                op=ALU.mult,
            )

    xq = []  # cross-image fill units (fin of prev image, qkT/v of next)
    pvq = []  # PV fill units; carries across images so the last slot's PV
    # matmuls interleave with the next image's first QK chunks

    def attention(im):
        nonlocal pvq
        st[im]["pvs"] = [
            pvsp.tile([D, S], MM, tag=f"pvs{h}", name=f"pvs_sb_{im}_{h}")
            for h in range(H)
        ]
        st[im]["den"] = {}
        st[im]["res"] = [
            resp.tile([P, S], MM, tag=f"res{k}", name=f"res_sb_{im}_{k}")
            for k in range(CT)
        ]
        slot_idx = 0
        for pi in range(H // 2):
            q_tile = st[im]["qk"][pi]
            k_tile = st[im]["qk"][QKT // 2 + pi]
            if pi not in st[im]["den"]:
                st[im]["den"][pi] = recp.tile(
                    [2 * SH, 512], f32, tag="den", name=f"den_{im}_{pi}"
                )
            for sh in range(SH):
                ex = expp.tile([P, TT * 2 * 512], MM, tag="exp")
                ex5 = ex[:].rearrange("p (t g s) -> p t g s", t=TT, g=2)
                for j in range(TT):
                    pl = ps_lg.tile([P, 1024], f32, tag="lg")
                    for hs in range(2):  # head A (rows 0:64), head B (64:128)
                        po = hs * D
                        nc.tensor.matmul(
                            pl[:, hs * 512 : (hs + 1) * 512],
                            k_tile[po : po + D, j * P : (j + 1) * P],
                            q_tile[po : po + D, sh * 512 : (sh + 1) * 512],
                            start=True,
                            stop=True,
                        )
                    nc.scalar.activation(
                        ex5[:, j, :, :],
                        pl[:].rearrange("p (g s) -> p g s", g=2),
                        AF.Exp,
                        scale=SCALE,
                    )
                    if pvq:
                        pvq.pop(0)()
                    elif xq and (slot_idx > 0 or j % 2 == 1):
                        # slot 0 of an image has no PV fills, so every chunk
                        # pop would hit xq and pace the very first exps at PE
                        # speed; fill only alternate chunks there
                        xq.pop(0)()
                    # one extra fill per chosen chunk, never two adjacent:
                    # ACT's lookahead backlog (~2 chunks) absorbs a single
                    # ~1.6us insertion but starves on anything larger
                    if xq and (
                        j in (2, 4, 6)
                        if slot_idx == 0
                        else (j in (1, 4, 6) if slot_idx < 5 else j in (1, 3, 5, 7))
                    ):
                        xq.pop(0)()
                if slot_idx == 0:
                    # finish the fills slot 0 skipped (v units must all be
                    # emitted before the PV units queued below consume them)
                    for _ in range(min(2, len(xq))):
                        xq.pop(0)()
                slot_idx += 1
                for hs in range(2):
                    pvq += pv_subunits(im, 2 * pi + hs, sh, ex5, hs)
        while pvq:
            pvq.pop(0)()

    # --- prologue: image 0 projections (qk tiles 0 and 4 first so attention
    # can start early; the rest plus v are needed before the first PV) ---
    emit_x(0)
    alloc_qk(0)
    alloc_v(0)
    for j in (0, QKT // 2):
        for sh in range(SH):
            qkT_unit(0, j, sh)()
    for j in range(TT):
        xq.append(v_unit(0, j))
    for j in (1, 5, 2, 6, 3, 7):
        for sh in range(SH):
            xq.append(qkT_unit(0, j, sh))

    for im in range(B_LOC):
        if im + 1 < B_LOC:
            emit_x(im + 1)
            alloc_qk(im + 1)
            alloc_v(im + 1)
            for j in (0, 4):
                for sh in range(SH):
                    xq.append(qkT_unit(im + 1, j, sh))
            for j in range(TT):
                xq.append(v_unit(im + 1, j))
            for j in (1, 5):
                for sh in range(SH):
                    xq.append(qkT_unit(im + 1, j, sh))
        attention(im)
        if im + 1 == B_LOC:
            while pvq:
                pvq.pop(0)()
            while xq:
                xq.pop(0)()
        if im + 1 < B_LOC:
            # pair-2/3 qkT of the next image drains inside its own early
            # slots (2 guaranteed pops per slot reach each pair in time)
            for j in (2, 6, 3, 7):
                for sh in range(SH):
                    xq.append(qkT_unit(im + 1, j, sh))
        if im + 1 < B_LOC:
            for j in range(CT):
                for sh in range(SH):
                    xq.append(fin_unit(im, j, sh))
        else:
            for j in range(CT):
                for sh in range(SH):
                    fin_unit(im, j, sh)()


_NC_CACHE = None
TRACE = False  # set True before calling kernel() to capture an NTFF profile
LAST_RESULT = None


def _get_nc():
    global _NC_CACHE
    if _NC_CACHE is None:
        _NC_CACHE = _build_nc()
    return _NC_CACHE


def kernel(x, W_proj, b_proj, W_out, b_out):
    x = np.ascontiguousarray(np.asarray(x, dtype=np.float32))
    W_proj = np.asarray(W_proj, dtype=np.float32)
    b_proj = np.asarray(b_proj, dtype=np.float32)
    W_out = np.ascontiguousarray(np.asarray(W_out, dtype=np.float32))
    b_out = np.ascontiguousarray(np.asarray(b_out, dtype=np.float32))

    w_qk, w_v, b_qk, b_v = prep_weights(W_proj, b_proj)

    xs = x.reshape(B, C, S)
    xs_mm = xs.astype(NP_MM)
    w_qk = w_qk.astype(NP_MM)
    w_v = w_v.astype(NP_MM)
    w_out_mm = W_out.astype(NP_MM)

    nc = _get_nc()
    in_maps = []
    for i in range(N_CORES):
        in_maps.append(
            {
                "xin": np.ascontiguousarray(xs_mm[i * B_LOC : (i + 1) * B_LOC]),
                "xinf": np.ascontiguousarray(xs[i * B_LOC : (i + 1) * B_LOC]),
                "wqk": w_qk,
                "wv": w_v,
                "wout": w_out_mm,
                "bqk": b_qk,
                "bv": b_v,
                "bout": b_out,
            }
        )
    res = run_bass_kernel_spmd(nc, in_maps, list(range(N_CORES)), trace=TRACE)
    global LAST_RESULT
    LAST_RESULT = res
    out = np.concatenate([res.results[i]["y"] for i in range(N_CORES)], axis=0)
    return out.reshape(B, C, 32, 32)


if __name__ == "__main__":
    rng = np.random.default_rng(0)
    ins = {
        "x": rng.standard_normal((B, C, 32, 32), dtype=np.float32),
        "W_proj": rng.standard_normal((C, 3 * INNER), dtype=np.float32) * C**-0.5,
        "b_proj": rng.standard_normal((3 * INNER,), dtype=np.float32) * 0.01,
        "W_out": rng.standard_normal((INNER, C), dtype=np.float32) * INNER**-0.5,
        "b_out": rng.standard_normal((C,), dtype=np.float32) * 0.01,
    }
    out = kernel(**ins)
    print(out.shape, out.dtype)
